# revision 54
# baseline (speedup 1.0000x reference)
"""DDiT block kernel for 8 Trainium2 NeuronCores.

Sharding: core = (batch b = core//2, seq half = core%2). Each core:
  - computes adaLN modulation for its batch (tiny matmuls)
  - LN1 + modulation for the FULL 2048 tokens of its batch (k/v need them)
  - q for its own 1024 tokens, k/v for all 2048 (redundant compute instead of
    a collective)
  - rotary, non-causal attention for its 1024 queries, out-proj, residual,
    LN2 + modulation, MLP, residual
All activations live in feature-on-partition ("transposed") layout so no
on-device transposes are needed. The host pre-transposes x / weights and
re-assembles the output. Projection/MLP matmuls run as float32r (full PE rate
at N>=256, ~fp32 accuracy); attention q/k/v/E are bf16.

Host-side input rotation trick: each core's xT has its OWN 1024 tokens in
columns 0:1024 and the other half in 1024:2048 (rotary tables rotated the
same way), so one SPMD program works for every core with no per-core offsets.
Softmax skips the running-max (scores are O(1) by construction: 0.02-scale
weights), so exp/sum are single-pass and the softmax denominator falls out of
the attn@V matmul via a ones-column prepended to V.
"""

import numpy as np
import sys

sys.path.insert(0, "/opt/trn_rl_repo")

B, S, D, H, DH = 4, 2048, 768, 12, 64
COND, MLP = 128, 3072
EPS = 1e-5
P = 128
SH = S // 2          # tokens per core (1024)
DK = D // P          # 6 feature chunks
MK = MLP // P        # 24 mlp chunks
N_CORES = 8

_prog_cache = {}

WS = 16.0          # fp8 weight scale (keeps 0.02-scale weights in e4m3 normals)


def _patch_act_tables():
    """Force exp+ln to co-reside (set 'natural_log_exp_and_others') so the
    ACT engine never thrashes table loads between softmax exp and the
    ln-based rstd/reciprocal during attention."""
    import concourse.bacc as bacc_mod
    import concourse.mybir as mybir
    from concourse.hw_specs import get_activation_tables as _orig
    if getattr(bacc_mod, "_act_tbl_patched", False):
        return
    EXP = mybir.ActivationFunctionType.Exp
    LN = mybir.ActivationFunctionType.Ln

    def _patched(arch):
        out = {}
        for name, fns in _orig(arch).items():
            if name != "natural_log_exp_and_others":
                fns = fns - {EXP, LN}
            out[name] = fns
        return out

    bacc_mod.get_activation_tables = _patched
    bacc_mod._act_tbl_patched = True


def _build_program():
    import concourse.tile as tile
    from concourse import bacc
    import concourse.mybir as mybir
    from contextlib import ExitStack

    _patch_act_tables()

    f32 = mybir.dt.float32
    f32r = mybir.dt.float32r
    bf16 = mybir.dt.bfloat16
    fp8 = mybir.dt.float8e4
    i16 = mybir.dt.int16
    i8 = mybir.dt.int8
    DR = mybir.MatmulPerfMode.DoubleRow
    AF = mybir.ActivationFunctionType
    OP = mybir.AluOpType

    nc = bacc.Bacc("TRN2", target_bir_lowering=False, debug=False,
                   enable_asserts=False, num_devices=N_CORES)

    # ---- DRAM I/O (per-core shapes) ----
    xT_d = nc.dram_tensor("xT", [D, S], f32, kind="ExternalInput").ap()
    xT16_d = nc.dram_tensor("xT16", [P, DK, S], bf16, kind="ExternalInput").ap()
    c_d = nc.dram_tensor("cT", [COND, 1], bf16, kind="ExternalInput").ap()
    cos_d = nc.dram_tensor("cos4", [P, S], bf16, kind="ExternalInput").ap()
    sin_d = nc.dram_tensor("sin4", [P, S], bf16, kind="ExternalInput").ap()
    wada_d = nc.dram_tensor("WadaT", [COND, 6 * D], bf16, kind="ExternalInput").ap()
    bada_d = nc.dram_tensor("badaT", [P, 36], f32, kind="ExternalInput").ap()
    ln1w_d = nc.dram_tensor("ln1wT", [P, DK], f32, kind="ExternalInput").ap()
    ln2w_d = nc.dram_tensor("ln2wT", [P, DK], f32, kind="ExternalInput").ap()
    wqk_d = nc.dram_tensor("WqkB", [2 * DK, P, DK, P], fp8, kind="ExternalInput").ap()
    wv_d = nc.dram_tensor("WvP", [DK // 2, P, 2, D], fp8, kind="ExternalInput").ap()
    wout_d = nc.dram_tensor("WoB", [DK, P, DK, P], fp8, kind="ExternalInput").ap()
    w1_d = nc.dram_tensor("W1B", [MK, P, DK, P], fp8, kind="ExternalInput").ap()
    b1_d = nc.dram_tensor("b1T", [P, MK], f32, kind="ExternalInput").ap()
    w2_d = nc.dram_tensor("W2B", [DK, P, MK, P], bf16, kind="ExternalInput").ap()
    b2_d = nc.dram_tensor("b2T", [P, DK], f32, kind="ExternalInput").ap()
    out_d = nc.dram_tensor("outT", [D, SH], f32, kind="ExternalOutput").ap()
    o_d = nc.dram_tensor("o_scratch", [D, SH], bf16).ap()   # internal spill

    xT3 = xT_d.rearrange("(a p) n -> p a n", p=P)          # [128, 6, 2048]

    with tile.TileContext(nc) as tc, ExitStack() as ctx:
        # ---- whole-program pools ----
        base = ctx.enter_context(tc.tile_pool(name="base", bufs=1))
        wpool = ctx.enter_context(tc.tile_pool(name="wpool", bufs=3))
        stat = ctx.enter_context(tc.tile_pool(name="stat", bufs=2))
        bcast = ctx.enter_context(tc.tile_pool(name="bcast", bufs=2))
        sqp = ctx.enter_context(tc.tile_pool(name="sqp", bufs=2))
        x16p = ctx.enter_context(tc.tile_pool(name="x16p", bufs=7))

        # ======== Phase A: adaLN modulation ========
        cT = base.tile([COND, 1], bf16, name="cT")
        nc.sync.dma_start(cT[:], c_d[:, :])
        ada = base.tile([P, 36], f32, name="ada")
        with tc.tile_pool(name="psE", bufs=2, space="PSUM") as psE:
            for j in range(36):
                wt = wpool.tile([COND, P], bf16, tag="wada", name="wada")
                nc.sync.dma_start(wt[:], wada_d[:, j * P:(j + 1) * P])
                ps = psE.tile([P, 1], f32, tag="mm", name="ps_ada")
                nc.tensor.matmul(ps[:], wt[:], cT[:], start=True, stop=True)
                nc.vector.tensor_copy(ada[:, j:j + 1], ps[:])
        badaT = base.tile([P, 36], f32, name="badaT")
        nc.sync.dma_start(badaT[:], bada_d[:, :])
        nc.vector.tensor_add(ada[:], ada[:], badaT[:])
        nc.vector.tensor_scalar_add(ada[:, 6:12], ada[:, 6:12], 1.0)
        nc.vector.tensor_scalar_add(ada[:, 24:30], ada[:, 24:30], 1.0)
        ln1s = base.tile([P, DK], f32, name="ln1s")
        ln2s = base.tile([P, DK], f32, name="ln2s")
        lw = base.tile([P, DK], f32, name="lnw1")
        nc.sync.dma_start(lw[:], ln1w_d[:, :])
        nc.vector.tensor_mul(ln1s[:], lw[:], ada[:, 6:12])
        lw2 = base.tile([P, DK], f32, name="lnw2")
        nc.sync.dma_start(lw2[:], ln2w_d[:, :])
        nc.vector.tensor_mul(ln2s[:], lw2[:], ada[:, 24:30])
        g1 = base.tile([P, DK], f32, name="g1")
        nc.vector.tensor_scalar_mul(g1[:], ada[:, 12:18], 1.0 / WS)

        ones = base.tile([P, 1], bf16, name="ones")
        nc.vector.memset(ones[:], 1.0)
        epsT = base.tile([1, 1], f32, name="epsT")
        nc.vector.memset(epsT[:], EPS)
        b1s = base.tile([P, MK], f32, name="b1s")
        nc.sync.dma_start(b1s[:], b1_d[:, :])
        b2s = base.tile([P, DK], f32, name="b2s")
        nc.sync.dma_start(b2s[:], b2_d[:, :])
        oTs = base.tile([P, DK, SH], fp8, name="oTs")
        cosT = base.tile([P, S], bf16, name="cosT")
        sinT = base.tile([P, S], bf16, name="sinT")
        xsb = base.tile([P, DK, SH], f32, name="xsb")
        woSB = base.tile([P, DK, DK, P], fp8, name="woSB")
        w1SB = base.tile([P, MK, DK, P], fp8, name="w1SB")

        def ln_block(psp, src, sl, scale_cols, shift_col0, dst, dst_sl,
                     src_is_bf16=False):
            """LayerNorm+modulate 512 columns `sl` of src [128,6,*] -> dst."""
            ps_s = psp.tile([1, 512], f32, tag="st_s", name="ps_s")
            ps_q = psp.tile([1, 512], f32, tag="st_q", name="ps_q")
            x16s = []
            for k in range(DK):
                if src_is_bf16:
                    x16 = src[:, k, sl]
                else:
                    x16 = x16p.tile([P, 512], bf16, tag="x16", name="x16")
                    nc.vector.tensor_copy(x16[:], src[:, k, sl])
                x16s.append(x16)
                nc.tensor.matmul(ps_s[:], ones[:], x16[:],
                                 start=(k == 0), stop=(k == DK - 1))
                sq = sqp.tile([P, 512], bf16, tag="sq", name="sq")
                nc.scalar.activation(sq[:], x16[:], AF.Square)
                nc.tensor.matmul(ps_q[:], ones[:], sq[:],
                                 start=(k == 0), stop=(k == DK - 1))
            mean = stat.tile([1, 512], f32, tag="mean", name="mean")
            nc.vector.tensor_scalar_mul(mean[:], ps_s[:], 1.0 / D)
            var = stat.tile([1, 512], f32, tag="var", name="var")
            nc.vector.tensor_scalar_mul(var[:], ps_q[:], 1.0 / D)
            aux = stat.tile([1, 512], f32, tag="aux", name="aux")
            nc.vector.tensor_mul(aux[:], mean[:], mean[:])
            nc.vector.tensor_sub(var[:], var[:], aux[:])
            # rstd = exp(-0.5*ln(var+eps)) entirely on ACT
            nc.scalar.activation(aux[:], var[:], AF.Ln, bias=epsT[:])
            auxb = stat.tile([1, 512], bf16, tag="auxb", name="auxb")
            nc.scalar.activation(auxb[:], aux[:], AF.Exp, scale=-0.5)
            meanb = stat.tile([1, 512], bf16, tag="meanb", name="meanb")
            nc.vector.tensor_copy(meanb[:], mean[:])
            A128 = bcast.tile([P, 512], bf16, tag="A128", name="A128")
            B128 = bcast.tile([P, 512], bf16, tag="B128", name="B128")
            nc.gpsimd.partition_broadcast(A128[:], auxb[:])
            nc.gpsimd.partition_broadcast(B128[:], meanb[:])
            for k in range(DK):
                t2 = sqp.tile([P, 512], bf16, tag="t2", name="t2")
                nc.vector.tensor_sub(t2[:], x16s[k][:], B128[:])
                nc.vector.tensor_mul(t2[:], t2[:], A128[:])
                nc.vector.tensor_scalar(
                    dst[:, k, dst_sl], t2[:],
                    scale_cols[:, k:k + 1], ada[:, shift_col0 + k:shift_col0 + k + 1],
                    OP.mult, OP.add)

        # ======== qkv outputs (live through attention) ========
        # kP[p]: heads 2p (rows 0:64) and 2p+1 (rows 64:128) share one tile.
        with tc.tile_pool(name="qkv_out", bufs=1) as qko:
            qT = [qko.tile([P, SH], bf16, name=f"qT{m}") for m in range(DK)]
            kP = [qko.tile([P, S], bf16, name=f"kP{m}") for m in range(DK)]
            # vA2[j]: v for key-blocks 2j,2j+1 paired on dim1 for DoubleRow
            # attnV. Per head 68 cols: 64 v dims, col 64 = ones(WS) for the
            # softmax denominator, cols 65:68 zero pad (816B dim1 stride %16).
            VW = DH + 4
            vA2 = [qko.tile([P, 2, H, VW], fp8, name=f"vA2{j}")
                   for j in range(S // P // 2)]
            for j in range(S // P // 2):
                nc.vector.memset(vA2[j][:, :, :, DH:VW], 0.0)

            # ==== Phase B+C: stream 1024-token superblocks (rope fused) ====
            if True:
                with tc.tile_pool(name="phbc", bufs=2) as phbc, \
                     tc.tile_pool(name="wvp", bufs=1) as wvp, \
                     tc.tile_pool(name="rope_p", bufs=2) as rp, \
                     tc.tile_pool(name="psLN", bufs=1, space="PSUM") as psLN, \
                     tc.tile_pool(name="psQ", bufs=4, space="PSUM") as psQ:
                    wv = [wvp.tile([P, 2, D], fp8, name=f"wv{g}")
                          for g in range(DK // 2)]
                    for g in range(DK // 2):
                        nc.sync.dma_start(wv[g][:], wv_d[g])
                    for b2 in range(S // SH):
                        hb = phbc.tile([P, DK, SH], fp8, tag="hb", name="hb")
                        for i in range(SH // 512):
                            c0 = b2 * SH + i * 512
                            xb = phbc.tile([P, DK, 512], bf16, tag="xb", name="xb")
                            nc.sync.dma_start(xb[:], xT16_d[:, :, c0:c0 + 512])
                            ln_block(psLN, xb, slice(0, 512),
                                     ln1s, 0, hb, slice(i * 512, i * 512 + 512),
                                     src_is_bf16=True)
                        if b2 == 0:
                            nc.sync.dma_start(cosT[:], cos_d[:, :])
                            nc.sync.dma_start(sinT[:], sin_d[:, :])
                        projs = [(1, DK)] if b2 == 1 else [(0, 0), (1, DK)]
                        for is_k, wblk0 in projs:
                            for m in range(DK):
                                w6 = wpool.tile([P, DK, P], fp8, tag="w6", name="w6")
                                nc.sync.dma_start(w6[:], wqk_d[wblk0 + m])
                                pss = [psQ.tile([P, 512], f32, tag="mm",
                                                name=f"ps_qk{i}") for i in range(2)]
                                for g in range(DK // 2):
                                    for i in range(2):
                                        nc.tensor.matmul(
                                            pss[i][:], w6[:, 2 * g:2 * g + 2, :],
                                            hb[:, 2 * g:2 * g + 2,
                                               i * 512:(i + 1) * 512],
                                            start=(g == 0), stop=(g == DK // 2 - 1),
                                            perf_mode=DR)
                                for i in range(2):
                                    csl = slice(b2 * SH + i * 512,
                                                b2 * SH + (i + 1) * 512)
                                    if not is_k:
                                        nc.scalar.copy(qT[m][:, csl],
                                                       pss[i][:])
                                    else:
                                        nc.scalar.copy(kP[m][:, csl],
                                                       pss[i][:])
                                # fused rotary for this m-block's 1024 columns
                                bsl = slice(b2 * SH, (b2 + 1) * SH)
                                t = qT[m] if not is_k else kP[m]
                                tsl = slice(0, SH) if not is_k else bsl
                                sw = rp.tile([P, SH], bf16, tag="swap",
                                             name="sw")
                                nc.sync.dma_start(sw[0:32, :], t[32:64, tsl])
                                nc.sync.dma_start(sw[32:64, :], t[0:32, tsl])
                                nc.sync.dma_start(sw[64:96, :], t[96:128, tsl])
                                nc.sync.dma_start(sw[96:128, :], t[64:96, tsl])
                                nc.vector.tensor_mul(t[:, tsl], t[:, tsl],
                                                     cosT[:, tsl])
                                nc.vector.tensor_mul(sw[:], sw[:], sinT[:, tsl])
                                nc.vector.tensor_add(t[:, tsl], t[:, tsl],
                                                     sw[:])
                        for t in range(SH // P):
                            tt = b2 * (SH // P) + t
                            ps1 = psQ.tile([P, 512], f32, tag="mm", name="ps_v1")
                            ps2 = psQ.tile([P, 512], f32, tag="mm", name="ps_v2")
                            for g in range(DK // 2):
                                lhs = hb[:, 2 * g:2 * g + 2, t * P:(t + 1) * P]
                                nc.tensor.matmul(ps1[:], lhs, wv[g][:, :, 0:512],
                                                 start=(g == 0),
                                                 stop=(g == DK // 2 - 1),
                                                 perf_mode=DR)
                                nc.tensor.matmul(ps2[:, 0:256], lhs,
                                                 wv[g][:, :, 512:768],
                                                 start=(g == 0),
                                                 stop=(g == DK // 2 - 1),
                                                 perf_mode=DR)
                            vsl = vA2[tt // 2][:, tt % 2]
                            nc.scalar.copy(
                                vsl[:, 0:8, 0:DH],
                                ps1[:].rearrange("p (h d) -> p h d", d=DH))
                            nc.scalar.copy(
                                vsl[:, 8:H, 0:DH],
                                ps2[:, 0:256].rearrange("p (h d) -> p h d", d=DH))
                            nc.vector.memset(vsl[:, :, DH:DH + 1], WS)

                # ==== Phase D: attention ====
                # qb-outer items; 2 row-tiled K=64 score MMs run concurrently;
                # ~40% of softmax exps computed on DVE via bf16 Schraudolph
                # (i16 = x*A + B, bitcast to bf16) to relieve the ACT engine.
                SCH_A = 11.541561 * 0.125 / (WS * WS)   # 2^3/ln2 * logit scale
                SCH_B = 55.66                           # 7*2^3 - RMS offset
                nc.sync.dma_start(xsb[:], xT3[:, :, 0:SH])
                nc.sync.dma_start(woSB[:],
                                  wout_d.rearrange("m p k c -> p m k c"))
                nc.sync.dma_start(w1SB[:],
                                  w1_d.rearrange("m p k c -> p m k c"))
                with tc.tile_pool(name="attnw", bufs=3) as aw, \
                     tc.tile_pool(name="attns", bufs=2) as asml:
                    with tc.tile_pool(name="psS", bufs=3, space="PSUM") as psS, \
                         tc.tile_pool(name="psO", bufs=2, space="PSUM") as psO:
                        KC, LOOK = S // P, 3
                        items = [(qb, p, kc) for qb in range(2)
                                 for p in range(H // 2) for kc in range(KC)]

                        def emit_S(qb, p, kc):
                            sg = psS.tile([P, 2, 512], f32, tag="sg", name="sg")
                            ksl = slice(kc * P, (kc + 1) * P)
                            qsl = slice(qb * 512, (qb + 1) * 512)
                            nc.tensor.matmul(sg[:, 0, :],
                                             kP[p][0:DH, ksl],
                                             qT[p][0:DH, qsl],
                                             start=True, stop=True)
                            nc.tensor.matmul(sg[:, 1, :],
                                             kP[p][DH:P, ksl],
                                             qT[p][DH:P, qsl],
                                             start=True, stop=True)
                            return sg

                        sg_q = {i: emit_S(*items[i]) for i in range(LOOK)}
                        oag, E2 = {}, None
                        for idx, (qb, p, kc) in enumerate(items):
                            sg = sg_q.pop(idx)
                            if kc % 2 == 0:
                                E2 = aw.tile([P, 2, 2, 512], fp8, tag="E2",
                                             name="E2")
                            dst = E2[:, kc % 2]
                            if idx % 2 == 1:
                                nc.vector.tensor_scalar(dst.bitcast(i8), sg[:],
                                                        SCH_A, SCH_B,
                                                        OP.mult, OP.add)
                            else:
                                nc.scalar.activation(dst, sg[:], AF.Exp,
                                                     scale=0.125 / (WS * WS))
                            if idx + LOOK < len(items):
                                sg_q[idx + LOOK] = emit_S(*items[idx + LOOK])
                            if kc % 2 == 1:
                                kcp = kc // 2
                                for hh in range(2):
                                    h = 2 * p + hh
                                    if kcp == 0:
                                        oag[hh] = psO.tile([VW, 512], f32,
                                                           tag="oa", name="oa")
                                    nc.tensor.matmul(oag[hh][:],
                                                     vA2[kcp][:, :, h, :],
                                                     E2[:, :, hh, :],
                                                     start=(kcp == 0),
                                                     stop=(kcp == KC // 2 - 1),
                                                     perf_mode=DR)
                            if kc == KC - 1:
                                qsl = slice(qb * 512, (qb + 1) * 512)
                                osb = [asml.tile([DH + 1, 512], f32,
                                                 tag=f"osb{hh}", name=f"osb{hh}")
                                       for hh in range(2)]
                                d2 = asml.tile([1, 1024], f32, tag="d2",
                                               name="d2")
                                for hh in range(2):
                                    nc.vector.tensor_copy(osb[hh][:],
                                                          oag[hh][0:DH + 1, :])
                                    nc.sync.dma_start(
                                        d2[:, hh * 512:(hh + 1) * 512],
                                        osb[hh][DH:DH + 1, :])
                                nc.scalar.activation(d2[:], d2[:], AF.Ln)
                                nc.scalar.activation(d2[:], d2[:], AF.Exp,
                                                     scale=-1.0)
                                for hh in range(2):
                                    off = hh * DH
                                    rb = asml.tile([DH, 512], f32, tag="rb",
                                                   name="rb")
                                    nc.gpsimd.partition_broadcast(
                                        rb[:], d2[:, hh * 512:(hh + 1) * 512])
                                    ot = asml.tile([DH, 512], fp8, tag="ot",
                                                   name="ot")
                                    nc.gpsimd.tensor_mul(ot[:],
                                                         osb[hh][0:DH, :],
                                                         rb[:])
                                    nc.sync.dma_start(
                                        oTs[off:off + DH, p, qsl], ot[:])

        # ======== Phase E..G: proj+residual, LN2, MLP ========
        with tc.tile_pool(name="mlp_ph", bufs=1) as mp, \
             tc.tile_pool(name="mlp_tmp", bufs=2) as mt, \
             tc.tile_pool(name="psLN2", bufs=2, space="PSUM") as psLN2, \
             tc.tile_pool(name="psM", bufs=2, space="PSUM") as psM:
            x1 = mp.tile([P, DK, SH], f32, name="x1")
            for m in range(DK):
                ps2 = psM.tile([P, 2, 512], f32, tag="mm2", name="ps_o2")
                for g in range(DK // 2):
                    for i in range(2):
                        nc.tensor.matmul(ps2[:, i, :],
                                         woSB[:, m, 2 * g:2 * g + 2, :],
                                         oTs[:, 2 * g:2 * g + 2,
                                             i * 512:(i + 1) * 512],
                                         start=(g == 0), stop=(g == DK // 2 - 1),
                                         perf_mode=DR)
                for i in range(2):
                    sl = slice(i * 512, i * 512 + 512)
                    nc.vector.scalar_tensor_tensor(
                        x1[:, m, sl], ps2[:, i, :], g1[:, m:m + 1],
                        xsb[:, m, sl], OP.mult, OP.add)

            h2 = mp.tile([P, DK, SH], fp8, name="h2")
            for i in range(SH // 512):
                sl = slice(i * 512, i * 512 + 512)
                ln_block(psLN2, x1, sl, ln2s, 18, h2, sl)

            m16 = mp.tile([P, MK, SH], bf16, name="m16")
            for m in range(MK):
                ps2g = psM.tile([P, 2, 512], f32, tag="mm2", name="ps_m2")
                for g in range(DK // 2):
                    for i in range(2):
                        nc.tensor.matmul(ps2g[:, i, :],
                                         w1SB[:, m, 2 * g:2 * g + 2, :],
                                         h2[:, 2 * g:2 * g + 2,
                                            i * 512:(i + 1) * 512],
                                         start=(g == 0), stop=(g == DK // 2 - 1),
                                         perf_mode=DR)
                nc.scalar.activation(m16[:, m, :], ps2g[:],
                                     AF.Gelu_apprx_tanh, bias=b1s[:, m:m + 1],
                                     scale=1.0 / WS)

            for m in range(DK):
                w24 = mt.tile([P, MK, P], bf16, tag="w24", name="w24")
                nc.sync.dma_start(w24[:], w2_d[m])
                ps2 = psM.tile([P, 2, 512], f32, tag="mm2", name="ps_y2")
                for k in range(MK):
                    for i in range(2):
                        nc.tensor.matmul(ps2[:, i, :], w24[:, k, :],
                                         m16[:, k, i * 512:(i + 1) * 512],
                                         start=(k == 0), stop=(k == MK - 1))
                for i in range(2):
                    sl = slice(i * 512, i * 512 + 512)
                    yt = mt.tile([P, 512], f32, tag="yt", name="yt")
                    nc.vector.tensor_scalar(yt[:], ps2[:, i, :], b2s[:, m:m + 1],
                                            ada[:, 30 + m:31 + m], OP.add, OP.mult)
                    nc.vector.tensor_add(yt[:], yt[:], x1[:, m, sl])
                    nc.sync.dma_start(out_d[m * P:(m + 1) * P, sl], yt[:])

    nc.compile()
    return nc


def _host_prep(inputs):
    """Build per-core in_maps (host-side sharding + layout transforms)."""
    import ml_dtypes
    bf16 = ml_dtypes.bfloat16
    fp8 = ml_dtypes.float8_e4m3

    x = np.ascontiguousarray(inputs["x"], dtype=np.float32)
    cos = np.asarray(inputs["cos"], dtype=np.float32)
    sin = np.asarray(inputs["sin"], dtype=np.float32)
    c = np.asarray(inputs["c"], dtype=np.float32)

    cos_s = cos[0, :, 0, 0, :DH // 2]      # (S, 32)
    sin_s = sin[0, :, 0, 0, :DH // 2]
    # C4[p, t] = cos_s[t, p%32]; S4 sign-folded: -sin for (p%64)<32 else +sin
    pidx = np.arange(P)
    C4 = cos_s.T[pidx % 32, :]             # (128, S)
    sgn = np.where((pidx % 64) < 32, -1.0, 1.0).astype(np.float32)
    S4 = sin_s.T[pidx % 32, :] * sgn[:, None]

    WadaT = np.ascontiguousarray(inputs["W_ada"].T).astype(bf16)        # (128, 4608)
    badaT = np.ascontiguousarray(
        np.asarray(inputs["b_ada"], np.float32).reshape(36, P).T)       # (128, 36)
    def blocks(wT, nblk):
        # wT: (K, N) -> (nblk, 128, K//128, 128): block m holds lhsT tiles
        K, N = wT.shape
        return np.ascontiguousarray(
            (wT * WS).reshape(K // P, P, nblk, P).transpose(2, 1, 0, 3)).astype(fp8)

    WqkvT = inputs["W_qkv"].T.astype(np.float32)                        # (768, 2304)
    WqkB = blocks(WqkvT[:, :2 * D], 2 * DK)                             # (12,128,6,128)
    # v weights: (768 contraction, 768 out) -> (3, 128, 2, 768) k-pair blocks
    WvP = np.ascontiguousarray(
        (WqkvT[:, 2 * D:] * WS).reshape(DK // 2, 2, P, D)
        .transpose(0, 2, 1, 3)).astype(fp8)
    WoB = blocks(inputs["W_out"].T.astype(np.float32), DK)
    W1B = blocks(inputs["W_mlp1"].T.astype(np.float32), MK)
    wT2 = inputs["W_mlp2"].T.astype(np.float32)                          # (3072, 768)
    W2B = np.ascontiguousarray(
        wT2.reshape(MK, P, DK, P).transpose(2, 1, 0, 3)).astype(bf16)
    b1T = np.ascontiguousarray(
        np.asarray(inputs["b_mlp1"], np.float32).reshape(MK, P).T)      # (128, 24)
    b2T = np.ascontiguousarray(
        np.asarray(inputs["b_mlp2"], np.float32).reshape(DK, P).T)      # (128, 6)
    ln1wT = np.ascontiguousarray(
        np.asarray(inputs["ln1_w"], np.float32).reshape(DK, P).T)       # (128, 6)
    ln2wT = np.ascontiguousarray(
        np.asarray(inputs["ln2_w"], np.float32).reshape(DK, P).T)

    in_maps = []
    for core in range(N_CORES):
        b, half = core // 2, core % 2
        own = slice(half * SH, half * SH + SH)
        oth = slice((1 - half) * SH, (1 - half) * SH + SH)
        xb = x[b]                                            # (S, D)
        xT = np.concatenate([xb[own].T, xb[oth].T], axis=1)  # (768, 2048) own first
        cos4 = np.concatenate([C4[:, own], C4[:, oth]], axis=1).astype(bf16)
        sin4 = np.concatenate([S4[:, own], S4[:, oth]], axis=1).astype(bf16)
        xT16 = np.ascontiguousarray(
            xT.reshape(DK, P, S).transpose(1, 0, 2)).astype(bf16)
        in_maps.append({
            "xT": np.ascontiguousarray(xT),
            "xT16": xT16,
            "cT": np.ascontiguousarray(c[b].reshape(COND, 1)).astype(bf16),
            "cos4": np.ascontiguousarray(cos4),
            "sin4": np.ascontiguousarray(sin4),
            "WadaT": WadaT, "badaT": badaT,
            "ln1wT": ln1wT, "ln2wT": ln2wT,
            "WqkB": WqkB, "WvP": WvP, "WoB": WoB,
            "W1B": W1B, "b1T": b1T, "W2B": W2B, "b2T": b2T,
        })
    return in_maps


def _get_program():
    if "nc" not in _prog_cache:
        _prog_cache["nc"] = _build_program()
    return _prog_cache["nc"]


def kernel(**inputs):
    from concourse.bass_utils import run_bass_kernel_spmd
    nc = _get_program()
    in_maps = _host_prep(inputs)
    res = run_bass_kernel_spmd(nc, in_maps, core_ids=list(range(N_CORES)))
    out = np.empty((B, S, D), dtype=np.float32)
    for core in range(N_CORES):
        b, half = core // 2, core % 2
        out[b, half * SH:(half + 1) * SH, :] = res.results[core]["outT"].T
    return out



# revision 55
# speedup vs baseline: 1.1819x; 1.1819x over previous
"""DDiT block kernel for 8 Trainium2 NeuronCores.

Sharding: core = (batch b = core//2, seq half = core%2). Each core:
  - computes adaLN modulation for its batch (tiny matmuls)
  - LN1 + modulation for the FULL 2048 tokens of its batch (k/v need them)
  - q for its own 1024 tokens, k/v for all 2048 (redundant compute instead of
    a collective)
  - rotary, non-causal attention for its 1024 queries, out-proj, residual,
    LN2 + modulation, MLP, residual
All activations live in feature-on-partition ("transposed") layout so no
on-device transposes are needed. The host pre-transposes x / weights and
re-assembles the output. Projection/MLP matmuls run as float32r (full PE rate
at N>=256, ~fp32 accuracy); attention q/k/v/E are bf16.

Host-side input rotation trick: each core's xT has its OWN 1024 tokens in
columns 0:1024 and the other half in 1024:2048 (rotary tables rotated the
same way), so one SPMD program works for every core with no per-core offsets.
Softmax skips the running-max (scores are O(1) by construction: 0.02-scale
weights), so exp/sum are single-pass and the softmax denominator falls out of
the attn@V matmul via a ones-column prepended to V.
"""

import numpy as np
import sys

sys.path.insert(0, "/opt/trn_rl_repo")

B, S, D, H, DH = 4, 2048, 768, 12, 64
COND, MLP = 128, 3072
EPS = 1e-5
P = 128
SH = S // 2          # tokens per core (1024)
DK = D // P          # 6 feature chunks
MK = MLP // P        # 24 mlp chunks
N_CORES = 8

_prog_cache = {}

WS = 16.0          # fp8 weight scale (keeps 0.02-scale weights in e4m3 normals)


def _patch_act_tables():
    """Force exp+ln to co-reside (set 'natural_log_exp_and_others') so the
    ACT engine never thrashes table loads between softmax exp and the
    ln-based rstd/reciprocal during attention."""
    import concourse.bacc as bacc_mod
    import concourse.mybir as mybir
    from concourse.hw_specs import get_activation_tables as _orig
    if getattr(bacc_mod, "_act_tbl_patched", False):
        return
    EXP = mybir.ActivationFunctionType.Exp
    LN = mybir.ActivationFunctionType.Ln

    def _patched(arch):
        out = {}
        for name, fns in _orig(arch).items():
            if name != "natural_log_exp_and_others":
                fns = fns - {EXP, LN}
            out[name] = fns
        return out

    bacc_mod.get_activation_tables = _patched
    bacc_mod._act_tbl_patched = True


def _build_program():
    import concourse.tile as tile
    from concourse import bacc
    import concourse.mybir as mybir
    from contextlib import ExitStack

    _patch_act_tables()

    f32 = mybir.dt.float32
    f32r = mybir.dt.float32r
    bf16 = mybir.dt.bfloat16
    fp8 = mybir.dt.float8e4
    i16 = mybir.dt.int16
    i8 = mybir.dt.int8
    DR = mybir.MatmulPerfMode.DoubleRow
    AF = mybir.ActivationFunctionType
    OP = mybir.AluOpType

    nc = bacc.Bacc("TRN2", target_bir_lowering=False, debug=False,
                   enable_asserts=False, num_devices=N_CORES)

    # ---- DRAM I/O (per-core shapes) ----
    xT_d = nc.dram_tensor("xT", [D, S], f32, kind="ExternalInput").ap()
    xT16_d = nc.dram_tensor("xT16", [P, DK, S], bf16, kind="ExternalInput").ap()
    c_d = nc.dram_tensor("cT", [COND, 1], bf16, kind="ExternalInput").ap()
    cos_d = nc.dram_tensor("cos4", [P, S], bf16, kind="ExternalInput").ap()
    sin_d = nc.dram_tensor("sin4", [P, S], bf16, kind="ExternalInput").ap()
    wada_d = nc.dram_tensor("WadaT", [COND, 6 * D], bf16, kind="ExternalInput").ap()
    bada_d = nc.dram_tensor("badaT", [P, 36], f32, kind="ExternalInput").ap()
    ln1w_d = nc.dram_tensor("ln1wT", [P, DK], f32, kind="ExternalInput").ap()
    ln2w_d = nc.dram_tensor("ln2wT", [P, DK], f32, kind="ExternalInput").ap()
    wqk_d = nc.dram_tensor("WqkB", [2 * DK, P, DK, P], fp8, kind="ExternalInput").ap()
    wv_d = nc.dram_tensor("WvP", [DK // 2, P, 2, D], fp8, kind="ExternalInput").ap()
    wout_d = nc.dram_tensor("WoB", [DK, P, DK, P], fp8, kind="ExternalInput").ap()
    w1_d = nc.dram_tensor("W1B", [MK, P, DK, P], fp8, kind="ExternalInput").ap()
    b1_d = nc.dram_tensor("b1T", [P, MK], f32, kind="ExternalInput").ap()
    w2_d = nc.dram_tensor("W2B", [DK, P, MK, P], bf16, kind="ExternalInput").ap()
    b2_d = nc.dram_tensor("b2T", [P, DK], f32, kind="ExternalInput").ap()
    out_d = nc.dram_tensor("outT", [D, SH], f32, kind="ExternalOutput").ap()
    o_d = nc.dram_tensor("o_scratch", [D, SH], bf16).ap()   # internal spill

    xT3 = xT_d.rearrange("(a p) n -> p a n", p=P)          # [128, 6, 2048]

    with tile.TileContext(nc) as tc, ExitStack() as ctx:
        # ---- whole-program pools ----
        base = ctx.enter_context(tc.tile_pool(name="base", bufs=1))
        wpool = ctx.enter_context(tc.tile_pool(name="wpool", bufs=3))
        stat = ctx.enter_context(tc.tile_pool(name="stat", bufs=2))
        bcast = ctx.enter_context(tc.tile_pool(name="bcast", bufs=2))
        sqp = ctx.enter_context(tc.tile_pool(name="sqp", bufs=2))
        x16p = ctx.enter_context(tc.tile_pool(name="x16p", bufs=7))

        # ======== Phase A: adaLN modulation ========
        cT = base.tile([COND, 1], bf16, name="cT")
        nc.sync.dma_start(cT[:], c_d[:, :])
        ada = base.tile([P, 36], f32, name="ada")
        with tc.tile_pool(name="psE", bufs=2, space="PSUM") as psE:
            for j in range(36):
                wt = wpool.tile([COND, P], bf16, tag="wada", name="wada")
                nc.sync.dma_start(wt[:], wada_d[:, j * P:(j + 1) * P])
                ps = psE.tile([P, 1], f32, tag="mm", name="ps_ada")
                nc.tensor.matmul(ps[:], wt[:], cT[:], start=True, stop=True)
                nc.vector.tensor_copy(ada[:, j:j + 1], ps[:])
        badaT = base.tile([P, 36], f32, name="badaT")
        nc.sync.dma_start(badaT[:], bada_d[:, :])
        nc.vector.tensor_add(ada[:], ada[:], badaT[:])
        nc.vector.tensor_scalar_add(ada[:, 6:12], ada[:, 6:12], 1.0)
        nc.vector.tensor_scalar_add(ada[:, 24:30], ada[:, 24:30], 1.0)
        ln1s = base.tile([P, DK], f32, name="ln1s")
        ln2s = base.tile([P, DK], f32, name="ln2s")
        lw = base.tile([P, DK], f32, name="lnw1")
        nc.sync.dma_start(lw[:], ln1w_d[:, :])
        nc.vector.tensor_mul(ln1s[:], lw[:], ada[:, 6:12])
        lw2 = base.tile([P, DK], f32, name="lnw2")
        nc.sync.dma_start(lw2[:], ln2w_d[:, :])
        nc.vector.tensor_mul(ln2s[:], lw2[:], ada[:, 24:30])
        g1 = base.tile([P, DK], f32, name="g1")
        nc.vector.tensor_scalar_mul(g1[:], ada[:, 12:18], 1.0 / WS)

        ones = base.tile([P, 1], bf16, name="ones")
        nc.vector.memset(ones[:], 1.0)
        epsT = base.tile([1, 1], f32, name="epsT")
        nc.vector.memset(epsT[:], EPS)
        b1s = base.tile([P, MK], f32, name="b1s")
        nc.sync.dma_start(b1s[:], b1_d[:, :])
        b2s = base.tile([P, DK], f32, name="b2s")
        nc.sync.dma_start(b2s[:], b2_d[:, :])
        oTs = base.tile([P, DK, SH], fp8, name="oTs")
        cosT = base.tile([P, S], bf16, name="cosT")
        sinT = base.tile([P, S], bf16, name="sinT")
        xsb = base.tile([P, DK, SH], f32, name="xsb")
        woSB = base.tile([P, DK, DK, P], fp8, name="woSB")
        w1SB = base.tile([P, MK, DK, P], fp8, name="w1SB")

        def ln_block(psp, src, sl, scale_cols, shift_col0, dst, dst_sl,
                     src_is_bf16=False):
            """LayerNorm+modulate 512 columns `sl` of src [128,6,*] -> dst."""
            ps_s = psp.tile([1, 512], f32, tag="st_s", name="ps_s")
            ps_q = psp.tile([1, 512], f32, tag="st_q", name="ps_q")
            x16s = []
            for k in range(DK):
                if src_is_bf16:
                    x16 = src[:, k, sl]
                else:
                    x16 = x16p.tile([P, 512], bf16, tag="x16", name="x16")
                    nc.vector.tensor_copy(x16[:], src[:, k, sl])
                x16s.append(x16)
                nc.tensor.matmul(ps_s[:], ones[:], x16[:],
                                 start=(k == 0), stop=(k == DK - 1))
                sq = sqp.tile([P, 512], bf16, tag="sq", name="sq")
                nc.scalar.activation(sq[:], x16[:], AF.Square)
                nc.tensor.matmul(ps_q[:], ones[:], sq[:],
                                 start=(k == 0), stop=(k == DK - 1))
            mean = stat.tile([1, 512], f32, tag="mean", name="mean")
            nc.vector.tensor_scalar_mul(mean[:], ps_s[:], 1.0 / D)
            var = stat.tile([1, 512], f32, tag="var", name="var")
            nc.vector.tensor_scalar_mul(var[:], ps_q[:], 1.0 / D)
            aux = stat.tile([1, 512], f32, tag="aux", name="aux")
            nc.vector.tensor_mul(aux[:], mean[:], mean[:])
            nc.vector.tensor_sub(var[:], var[:], aux[:])
            # rstd = exp(-0.5*ln(var+eps)) entirely on ACT
            nc.scalar.activation(aux[:], var[:], AF.Ln, bias=epsT[:])
            auxb = stat.tile([1, 512], bf16, tag="auxb", name="auxb")
            nc.scalar.activation(auxb[:], aux[:], AF.Exp, scale=-0.5)
            meanb = stat.tile([1, 512], bf16, tag="meanb", name="meanb")
            nc.vector.tensor_copy(meanb[:], mean[:])
            A128 = bcast.tile([P, 512], bf16, tag="A128", name="A128")
            B128 = bcast.tile([P, 512], bf16, tag="B128", name="B128")
            nc.gpsimd.partition_broadcast(A128[:], auxb[:])
            nc.gpsimd.partition_broadcast(B128[:], meanb[:])
            for k in range(DK):
                t2 = sqp.tile([P, 512], bf16, tag="t2", name="t2")
                nc.vector.tensor_sub(t2[:], x16s[k][:], B128[:])
                nc.vector.tensor_mul(t2[:], t2[:], A128[:])
                nc.vector.tensor_scalar(
                    dst[:, k, dst_sl], t2[:],
                    scale_cols[:, k:k + 1], ada[:, shift_col0 + k:shift_col0 + k + 1],
                    OP.mult, OP.add)

        # ======== qkv outputs (live through attention) ========
        # kP[p]: heads 2p (rows 0:64) and 2p+1 (rows 64:128) share one tile.
        with tc.tile_pool(name="qkv_out", bufs=1) as qko:
            qT = [qko.tile([P, SH], bf16, name=f"qT{m}") for m in range(DK)]
            kP = [qko.tile([P, S], bf16, name=f"kP{m}") for m in range(DK)]
            # vA2[j]: v for key-blocks 2j,2j+1 paired on dim1 for DoubleRow
            # attnV. Per head 68 cols: 64 v dims, col 64 = ones(WS) for the
            # softmax denominator, cols 65:68 zero pad (816B dim1 stride %16).
            VW = DH + 4
            vA2 = [qko.tile([P, 2, H, VW], fp8, name=f"vA2{j}")
                   for j in range(S // P // 2)]
            for j in range(S // P // 2):
                nc.vector.memset(vA2[j][:, :, :, DH:VW], 0.0)

            # ==== Phase B+C: stream 1024-token superblocks (rope fused) ====
            if True:
                with tc.tile_pool(name="phbc", bufs=2) as phbc, \
                     tc.tile_pool(name="wvp", bufs=1) as wvp, \
                     tc.tile_pool(name="rope_p", bufs=2) as rp, \
                     tc.tile_pool(name="psLN", bufs=1, space="PSUM") as psLN, \
                     tc.tile_pool(name="psQ", bufs=4, space="PSUM") as psQ:
                    wv = [wvp.tile([P, 2, D], fp8, name=f"wv{g}")
                          for g in range(DK // 2)]
                    for g in range(DK // 2):
                        nc.sync.dma_start(wv[g][:], wv_d[g])
                    for b2 in range(S // SH):
                        hb = phbc.tile([P, DK, SH], fp8, tag="hb", name="hb")
                        for i in range(SH // 512):
                            c0 = b2 * SH + i * 512
                            xb = phbc.tile([P, DK, 512], bf16, tag="xb", name="xb")
                            nc.sync.dma_start(xb[:], xT16_d[:, :, c0:c0 + 512])
                            ln_block(psLN, xb, slice(0, 512),
                                     ln1s, 0, hb, slice(i * 512, i * 512 + 512),
                                     src_is_bf16=True)
                        if b2 == 0:
                            nc.sync.dma_start(cosT[:], cos_d[:, :])
                            nc.sync.dma_start(sinT[:], sin_d[:, :])
                        projs = [(1, DK)] if b2 == 1 else [(0, 0), (1, DK)]
                        for is_k, wblk0 in projs:
                            for m in range(DK):
                                w6 = wpool.tile([P, DK, P], fp8, tag="w6", name="w6")
                                nc.sync.dma_start(w6[:], wqk_d[wblk0 + m])
                                pss = [psQ.tile([P, 512], f32, tag="mm",
                                                name=f"ps_qk{i}") for i in range(2)]
                                for g in range(DK // 2):
                                    for i in range(2):
                                        nc.tensor.matmul(
                                            pss[i][:], w6[:, 2 * g:2 * g + 2, :],
                                            hb[:, 2 * g:2 * g + 2,
                                               i * 512:(i + 1) * 512],
                                            start=(g == 0), stop=(g == DK // 2 - 1),
                                            perf_mode=DR)
                                for i in range(2):
                                    csl = slice(b2 * SH + i * 512,
                                                b2 * SH + (i + 1) * 512)
                                    if not is_k:
                                        nc.scalar.copy(qT[m][:, csl],
                                                       pss[i][:])
                                    else:
                                        nc.scalar.copy(kP[m][:, csl],
                                                       pss[i][:])
                                # fused rotary for this m-block's 1024 columns
                                bsl = slice(b2 * SH, (b2 + 1) * SH)
                                t = qT[m] if not is_k else kP[m]
                                tsl = slice(0, SH) if not is_k else bsl
                                sw = rp.tile([P, SH], bf16, tag="swap",
                                             name="sw")
                                nc.scalar.dma_start(sw[0:32, :], t[32:64, tsl])
                                nc.scalar.dma_start(sw[32:64, :], t[0:32, tsl])
                                nc.scalar.dma_start(sw[64:96, :], t[96:128, tsl])
                                nc.scalar.dma_start(sw[96:128, :], t[64:96, tsl])
                                nc.vector.tensor_mul(t[:, tsl], t[:, tsl],
                                                     cosT[:, tsl])
                                nc.vector.tensor_mul(sw[:], sw[:], sinT[:, tsl])
                                nc.vector.tensor_add(t[:, tsl], t[:, tsl],
                                                     sw[:])
                        for t in range(SH // P):
                            tt = b2 * (SH // P) + t
                            ps1 = psQ.tile([P, 512], f32, tag="mm", name="ps_v1")
                            ps2 = psQ.tile([P, 512], f32, tag="mm", name="ps_v2")
                            for g in range(DK // 2):
                                lhs = hb[:, 2 * g:2 * g + 2, t * P:(t + 1) * P]
                                nc.tensor.matmul(ps1[:], lhs, wv[g][:, :, 0:512],
                                                 start=(g == 0),
                                                 stop=(g == DK // 2 - 1),
                                                 perf_mode=DR)
                                nc.tensor.matmul(ps2[:, 0:256], lhs,
                                                 wv[g][:, :, 512:768],
                                                 start=(g == 0),
                                                 stop=(g == DK // 2 - 1),
                                                 perf_mode=DR)
                            vsl = vA2[tt // 2][:, tt % 2]
                            nc.scalar.copy(
                                vsl[:, 0:8, 0:DH],
                                ps1[:].rearrange("p (h d) -> p h d", d=DH))
                            nc.scalar.copy(
                                vsl[:, 8:H, 0:DH],
                                ps2[:, 0:256].rearrange("p (h d) -> p h d", d=DH))
                            nc.vector.memset(vsl[:, :, DH:DH + 1], WS)

                # ==== Phase D: attention ====
                # qb-outer items; 2 row-tiled K=64 score MMs run concurrently;
                # ~40% of softmax exps computed on DVE via bf16 Schraudolph
                # (i16 = x*A + B, bitcast to bf16) to relieve the ACT engine.
                SCH_A = 11.541561 * 0.125 / (WS * WS)   # 2^3/ln2 * logit scale
                SCH_B = 55.66                           # 7*2^3 - RMS offset
                nc.sync.dma_start(xsb[:], xT3[:, :, 0:SH])
                nc.sync.dma_start(woSB[:],
                                  wout_d.rearrange("m p k c -> p m k c"))
                nc.sync.dma_start(w1SB[:],
                                  w1_d.rearrange("m p k c -> p m k c"))
                with tc.tile_pool(name="attnw", bufs=3) as aw, \
                     tc.tile_pool(name="attns", bufs=2) as asml:
                    with tc.tile_pool(name="psS", bufs=3, space="PSUM") as psS, \
                         tc.tile_pool(name="psO", bufs=2, space="PSUM") as psO:
                        KC, LOOK = S // P, 3
                        items = [(qb, p, kc) for qb in range(2)
                                 for p in range(H // 2) for kc in range(KC)]

                        def emit_S(qb, p, kc):
                            sg = psS.tile([P, 2, 512], f32, tag="sg", name="sg")
                            ksl = slice(kc * P, (kc + 1) * P)
                            qsl = slice(qb * 512, (qb + 1) * 512)
                            nc.tensor.matmul(sg[:, 0, :],
                                             kP[p][0:DH, ksl],
                                             qT[p][0:DH, qsl],
                                             start=True, stop=True)
                            nc.tensor.matmul(sg[:, 1, :],
                                             kP[p][DH:P, ksl],
                                             qT[p][DH:P, qsl],
                                             start=True, stop=True)
                            return sg

                        sg_q = {i: emit_S(*items[i]) for i in range(LOOK)}
                        oag, E2 = {}, None
                        for idx, (qb, p, kc) in enumerate(items):
                            sg = sg_q.pop(idx)
                            if kc % 2 == 0:
                                E2 = aw.tile([P, 2, 2, 512], fp8, tag="E2",
                                             name="E2")
                            dst = E2[:, kc % 2]
                            if idx % 5 in (1, 3):
                                nc.vector.tensor_scalar(dst.bitcast(i8), sg[:],
                                                        SCH_A, SCH_B,
                                                        OP.mult, OP.add)
                            else:
                                nc.scalar.activation(dst, sg[:], AF.Exp,
                                                     scale=0.125 / (WS * WS))
                            if idx + LOOK < len(items):
                                sg_q[idx + LOOK] = emit_S(*items[idx + LOOK])
                            if kc % 2 == 1:
                                kcp = kc // 2
                                for hh in range(2):
                                    h = 2 * p + hh
                                    if kcp == 0:
                                        oag[hh] = psO.tile([VW, 512], f32,
                                                           tag="oa", name="oa")
                                    nc.tensor.matmul(oag[hh][:],
                                                     vA2[kcp][:, :, h, :],
                                                     E2[:, :, hh, :],
                                                     start=(kcp == 0),
                                                     stop=(kcp == KC // 2 - 1),
                                                     perf_mode=DR)
                            if kc == KC - 1:
                                qsl = slice(qb * 512, (qb + 1) * 512)
                                osb = [asml.tile([DH + 1, 512], f32,
                                                 tag=f"osb{hh}", name=f"osb{hh}")
                                       for hh in range(2)]
                                d2 = asml.tile([1, 1024], f32, tag="d2",
                                               name="d2")
                                for hh in range(2):
                                    nc.vector.tensor_copy(osb[hh][:],
                                                          oag[hh][0:DH + 1, :])
                                    nc.sync.dma_start(
                                        d2[:, hh * 512:(hh + 1) * 512],
                                        osb[hh][DH:DH + 1, :])
                                nc.scalar.activation(d2[:], d2[:], AF.Ln)
                                nc.scalar.activation(d2[:], d2[:], AF.Exp,
                                                     scale=-1.0)
                                for hh in range(2):
                                    off = hh * DH
                                    rb = asml.tile([DH, 512], f32, tag="rb",
                                                   name="rb")
                                    nc.gpsimd.partition_broadcast(
                                        rb[:], d2[:, hh * 512:(hh + 1) * 512])
                                    ot = asml.tile([DH, 512], fp8, tag="ot",
                                                   name="ot")
                                    nc.vector.tensor_mul(ot[:],
                                                         osb[hh][0:DH, :],
                                                         rb[:])
                                    nc.sync.dma_start(
                                        oTs[off:off + DH, p, qsl], ot[:])

        # ======== Phase E..G: proj+residual, LN2, MLP ========
        with tc.tile_pool(name="mlp_ph", bufs=1) as mp, \
             tc.tile_pool(name="mlp_tmp", bufs=2) as mt, \
             tc.tile_pool(name="psLN2", bufs=2, space="PSUM") as psLN2, \
             tc.tile_pool(name="psM", bufs=2, space="PSUM") as psM:
            x1 = mp.tile([P, DK, SH], f32, name="x1")
            for m in range(DK):
                ps2 = psM.tile([P, 2, 512], f32, tag="mm2", name="ps_o2")
                for g in range(DK // 2):
                    for i in range(2):
                        nc.tensor.matmul(ps2[:, i, :],
                                         woSB[:, m, 2 * g:2 * g + 2, :],
                                         oTs[:, 2 * g:2 * g + 2,
                                             i * 512:(i + 1) * 512],
                                         start=(g == 0), stop=(g == DK // 2 - 1),
                                         perf_mode=DR)
                for i in range(2):
                    sl = slice(i * 512, i * 512 + 512)
                    nc.vector.scalar_tensor_tensor(
                        x1[:, m, sl], ps2[:, i, :], g1[:, m:m + 1],
                        xsb[:, m, sl], OP.mult, OP.add)

            h2 = mp.tile([P, DK, SH], fp8, name="h2")
            for i in range(SH // 512):
                sl = slice(i * 512, i * 512 + 512)
                ln_block(psLN2, x1, sl, ln2s, 18, h2, sl)

            m16 = mp.tile([P, MK, SH], bf16, name="m16")
            for m in range(MK):
                ps2g = psM.tile([P, 2, 512], f32, tag="mm2", name="ps_m2")
                for g in range(DK // 2):
                    for i in range(2):
                        nc.tensor.matmul(ps2g[:, i, :],
                                         w1SB[:, m, 2 * g:2 * g + 2, :],
                                         h2[:, 2 * g:2 * g + 2,
                                            i * 512:(i + 1) * 512],
                                         start=(g == 0), stop=(g == DK // 2 - 1),
                                         perf_mode=DR)
                nc.scalar.activation(m16[:, m, :], ps2g[:],
                                     AF.Gelu_apprx_tanh, bias=b1s[:, m:m + 1],
                                     scale=1.0 / WS)

            for m in range(DK):
                w24 = mt.tile([P, MK, P], bf16, tag="w24", name="w24")
                nc.sync.dma_start(w24[:], w2_d[m])
                ps2 = psM.tile([P, 2, 512], f32, tag="mm2", name="ps_y2")
                for k in range(MK):
                    for i in range(2):
                        nc.tensor.matmul(ps2[:, i, :], w24[:, k, :],
                                         m16[:, k, i * 512:(i + 1) * 512],
                                         start=(k == 0), stop=(k == MK - 1))
                for i in range(2):
                    sl = slice(i * 512, i * 512 + 512)
                    yt = mt.tile([P, 512], f32, tag="yt", name="yt")
                    nc.vector.tensor_scalar(yt[:], ps2[:, i, :], b2s[:, m:m + 1],
                                            ada[:, 30 + m:31 + m], OP.add, OP.mult)
                    nc.vector.tensor_add(yt[:], yt[:], x1[:, m, sl])
                    nc.sync.dma_start(out_d[m * P:(m + 1) * P, sl], yt[:])

    nc.compile()
    return nc


def _host_prep(inputs):
    """Build per-core in_maps (host-side sharding + layout transforms)."""
    import ml_dtypes
    bf16 = ml_dtypes.bfloat16
    fp8 = ml_dtypes.float8_e4m3

    x = np.ascontiguousarray(inputs["x"], dtype=np.float32)
    cos = np.asarray(inputs["cos"], dtype=np.float32)
    sin = np.asarray(inputs["sin"], dtype=np.float32)
    c = np.asarray(inputs["c"], dtype=np.float32)

    cos_s = cos[0, :, 0, 0, :DH // 2]      # (S, 32)
    sin_s = sin[0, :, 0, 0, :DH // 2]
    # C4[p, t] = cos_s[t, p%32]; S4 sign-folded: -sin for (p%64)<32 else +sin
    pidx = np.arange(P)
    C4 = cos_s.T[pidx % 32, :]             # (128, S)
    sgn = np.where((pidx % 64) < 32, -1.0, 1.0).astype(np.float32)
    S4 = sin_s.T[pidx % 32, :] * sgn[:, None]

    WadaT = np.ascontiguousarray(inputs["W_ada"].T).astype(bf16)        # (128, 4608)
    badaT = np.ascontiguousarray(
        np.asarray(inputs["b_ada"], np.float32).reshape(36, P).T)       # (128, 36)
    def blocks(wT, nblk):
        # wT: (K, N) -> (nblk, 128, K//128, 128): block m holds lhsT tiles
        K, N = wT.shape
        return np.ascontiguousarray(
            (wT * WS).reshape(K // P, P, nblk, P).transpose(2, 1, 0, 3)).astype(fp8)

    WqkvT = inputs["W_qkv"].T.astype(np.float32)                        # (768, 2304)
    WqkB = blocks(WqkvT[:, :2 * D], 2 * DK)                             # (12,128,6,128)
    # v weights: (768 contraction, 768 out) -> (3, 128, 2, 768) k-pair blocks
    WvP = np.ascontiguousarray(
        (WqkvT[:, 2 * D:] * WS).reshape(DK // 2, 2, P, D)
        .transpose(0, 2, 1, 3)).astype(fp8)
    WoB = blocks(inputs["W_out"].T.astype(np.float32), DK)
    W1B = blocks(inputs["W_mlp1"].T.astype(np.float32), MK)
    wT2 = inputs["W_mlp2"].T.astype(np.float32)                          # (3072, 768)
    W2B = np.ascontiguousarray(
        wT2.reshape(MK, P, DK, P).transpose(2, 1, 0, 3)).astype(bf16)
    b1T = np.ascontiguousarray(
        np.asarray(inputs["b_mlp1"], np.float32).reshape(MK, P).T)      # (128, 24)
    b2T = np.ascontiguousarray(
        np.asarray(inputs["b_mlp2"], np.float32).reshape(DK, P).T)      # (128, 6)
    ln1wT = np.ascontiguousarray(
        np.asarray(inputs["ln1_w"], np.float32).reshape(DK, P).T)       # (128, 6)
    ln2wT = np.ascontiguousarray(
        np.asarray(inputs["ln2_w"], np.float32).reshape(DK, P).T)

    in_maps = []
    for core in range(N_CORES):
        b, half = core // 2, core % 2
        own = slice(half * SH, half * SH + SH)
        oth = slice((1 - half) * SH, (1 - half) * SH + SH)
        xb = x[b]                                            # (S, D)
        xT = np.concatenate([xb[own].T, xb[oth].T], axis=1)  # (768, 2048) own first
        cos4 = np.concatenate([C4[:, own], C4[:, oth]], axis=1).astype(bf16)
        sin4 = np.concatenate([S4[:, own], S4[:, oth]], axis=1).astype(bf16)
        xT16 = np.ascontiguousarray(
            xT.reshape(DK, P, S).transpose(1, 0, 2)).astype(bf16)
        in_maps.append({
            "xT": np.ascontiguousarray(xT),
            "xT16": xT16,
            "cT": np.ascontiguousarray(c[b].reshape(COND, 1)).astype(bf16),
            "cos4": np.ascontiguousarray(cos4),
            "sin4": np.ascontiguousarray(sin4),
            "WadaT": WadaT, "badaT": badaT,
            "ln1wT": ln1wT, "ln2wT": ln2wT,
            "WqkB": WqkB, "WvP": WvP, "WoB": WoB,
            "W1B": W1B, "b1T": b1T, "W2B": W2B, "b2T": b2T,
        })
    return in_maps


def _get_program():
    if "nc" not in _prog_cache:
        _prog_cache["nc"] = _build_program()
    return _prog_cache["nc"]


def kernel(**inputs):
    from concourse.bass_utils import run_bass_kernel_spmd
    nc = _get_program()
    in_maps = _host_prep(inputs)
    res = run_bass_kernel_spmd(nc, in_maps, core_ids=list(range(N_CORES)))
    out = np.empty((B, S, D), dtype=np.float32)
    for core in range(N_CORES):
        b, half = core // 2, core % 2
        out[b, half * SH:(half + 1) * SH, :] = res.results[core]["outT"].T
    return out

